# revision 1
# baseline (speedup 1.0000x reference)
"""ALayer kernel for 8 TRN2 NeuronCores — pure data parallel over batch.

Per-core shard: 4 images of [256, 56, 56].
  h  = relu(conv3x3(x_in, w1))      # 256 -> 16 ch
  A  = sigmoid(conv3x3(h, w2))      # 16 -> 1 ch
  out = x_out * box3x3(A)           # broadcast over 256 ch

TensorEngine formulation (bf16 matmuls, fp32 PSUM accumulation):
  conv1: 18 accumulating shift-matmuls (2 K-chunks of 128 in-ch x 9 taps,
         M=16 out-ch) over zero-padded 58x58 planes, 14 output rows per tile.
  conv2: relu(h) is stored at 3 dx-shifted partition groups (H3, bases
         0/32/64), so conv2 is 3 accumulating K=96 matmuls (one per dy).
  box+broadcast: sigmoid output A is stored at 3 dx-shifted partition bases
         (A3); 3 accumulating matmuls with a rows-0/32/64-ones lhsT produce
         box3x3(A) replicated to 128 partitions directly in PSUM.
  final: DVE multiply of x_out by the broadcast PSUM tile.
"""

import numpy as np
import ml_dtypes

import concourse.bass as bass
import concourse.tile as tile
import concourse.mybir as mybir
from concourse import bacc
from concourse.bass_utils import run_bass_kernel_spmd

BF16 = mybir.dt.bfloat16
FP8 = mybir.dt.float8e4
F32 = mybir.dt.float32

B, C, H, W = 32, 256, 56, 56
NCORES = 8
BL = B // NCORES          # images per core
KCH = 2                   # 256 = 2 chunks of 128
HP = H + 2                # padded plane side
HW = H * W                # 3136
RG = 7                    # row groups per image
RROWS = H // RG           # 8 rows per group
NT = RROWS * W            # 448 px per tile

_cache = {}


def _build():
    nc = bacc.Bacc("TRN2", target_bir_lowering=False, debug=False)

    xin_d = nc.dram_tensor("xin", [BL, KCH, 128, HP * HP], FP8, kind="ExternalInput").ap()
    xout_d = nc.dram_tensor("xout", [BL, KCH, 128, HW], BF16, kind="ExternalInput").ap()
    w1_d = nc.dram_tensor("w1t", [9, 128, KCH, 16], FP8, kind="ExternalInput").ap()
    w2_d = nc.dram_tensor("w2t", [96, 3], BF16, kind="ExternalInput").ap()
    out_d = nc.dram_tensor("out", [BL, KCH, 128, HW], F32, kind="ExternalOutput").ap()

    with tile.TileContext(nc) as tc:
        with (
            tc.tile_pool(name="const", bufs=1) as constp,
            tc.tile_pool(name="xpad", bufs=4) as xpadp,
            tc.tile_pool(name="h3", bufs=3) as h3p,
            tc.tile_pool(name="a3", bufs=3) as a3p,
            tc.tile_pool(name="xo", bufs=2) as xop,
            tc.tile_pool(name="ot", bufs=2) as otp,
            tc.tile_pool(name="ps_h", bufs=2, space="PSUM") as ps_h,
            tc.tile_pool(name="ps_a", bufs=3, space="PSUM") as ps_a,
            tc.tile_pool(name="ps_b", bufs=3, space="PSUM") as ps_b,
        ):
            # weights (issued on the scalar queue so xin DMAs go first on sync)
            w1sb = constp.tile([128, 9, KCH, 16], FP8)
            w2sb = constp.tile([96, 3], BF16)
            nc.scalar.dma_start(w1sb[:], w1_d.transpose([1, 0, 2, 3]))
            nc.scalar.dma_start(w2sb[:], w2_d[:])
            # lhsT for fused box+broadcast: rows 0/32/64 ones, rest zero
            ones3 = constp.tile([96, 128], BF16)
            nc.vector.memset(ones3[:], 0.0)
            for j in range(3):
                nc.vector.memset(ones3[32 * j : 32 * j + 1, :], 1.0)

            xpads, h3s, a3s, xos, ots = {}, {}, {}, {}, {}

            def stage_front(img):
                # ---- load x_in (pre-padded fp8) ----
                xpad = xpadp.tile([128, KCH, HP, HP], FP8)
                xpads[img] = xpad
                xpf = xpad.rearrange("p k r w -> p k (r w)")
                MIDP = 29 * HP
                for hh in range(2):
                    sl = slice(hh * MIDP, HP * HP if hh else MIDP)
                    for k in range(KCH):
                        nc.sync.dma_start(xpf[:, k, sl], xin_d[img, k, :, sl])

                # ---- conv1 (+relu) -> H3 ----
                h3 = h3p.tile([96, HP, HP], BF16)
                h3s[img] = h3
                if img < 3:
                    nc.gpsimd.memset(h3[:], 0.0)
                for rg in range(RG):
                    r0 = 1 + rg * RROWS
                    hps = ps_h.tile([16, NT], F32)
                    for t in range(9):
                        dy, dx = t // 3 - 1, t % 3 - 1
                        nc.tensor.matmul(
                            hps[:],
                            w1sb[:, t, :, :],
                            xpad[:, :, r0 + dy : r0 + dy + RROWS, 1 + dx : 1 + dx + W],
                            start=(t == 0),
                            stop=(t == 8),
                            perf_mode=mybir.MatmulPerfMode.DoubleRow,
                        )
                    nc.scalar.activation(
                        h3[32:48, r0 : r0 + RROWS, 1 : 1 + W],
                        hps.rearrange("p (r w) -> p r w", r=RROWS),
                        mybir.ActivationFunctionType.Relu,
                    )
                # flat whole-plane shifted copies
                h3f = h3.rearrange("p r w -> p (r w)")
                PL = HP * HP
                MID = (PL // 2) & ~1
                nc.vector.tensor_copy(h3f[0:16, 1:MID], h3f[32:48, 0 : MID - 1])
                nc.vector.tensor_copy(h3f[64:80, 0 : MID - 1], h3f[32:48, 1:MID])
                nc.vector.tensor_copy(h3f[0:16, MID:PL], h3f[32:48, MID - 1 : PL - 1])
                nc.vector.tensor_copy(h3f[64:80, MID - 1 : PL - 1], h3f[32:48, MID:PL])

                # prefetch x_out
                xo = xop.tile([128, KCH, HW], BF16)
                xos[img] = xo
                for k in range(KCH):
                    nc.gpsimd.dma_start(xo[:, k, :], xout_d[img, k, :, :])

            def stage_back(img):
                h3 = h3s[img]
                # ---- conv2 + sigmoid -> A3 ----
                a3 = a3p.tile([96, HP, HP], BF16)
                if img < 3:
                    nc.gpsimd.memset(a3[:], 0.0)
                for rg in range(RG):
                    r0 = 1 + rg * RROWS
                    aps = ps_a.tile([1, NT], F32)
                    for d in range(3):
                        dy = d - 1
                        nc.tensor.matmul(
                            aps[:],
                            w2sb[:, d : d + 1],
                            h3[:, r0 + dy : r0 + dy + RROWS, 1 : 1 + W],
                            start=(d == 0),
                            stop=(d == 2),
                        )
                    nc.scalar.activation(
                        a3[32:33, r0 : r0 + RROWS, 1 : 1 + W],
                        aps.rearrange("p (r w) -> p r w", r=RROWS),
                        mybir.ActivationFunctionType.Sigmoid,
                    )
                a3f = a3.rearrange("p r w -> p (r w)")
                nc.vector.tensor_copy(a3f[0:1, 1 : HP * HP], a3f[32:33, 0 : HP * HP - 1])
                nc.vector.tensor_copy(a3f[64:65, 0 : HP * HP - 1], a3f[32:33, 1 : HP * HP])

                # ---- box3x3 + broadcast, multiply with x_out, store ----
                xo = xos[img]
                ot = otp.tile([128, KCH, HW], F32)
                for rg in range(RG):
                    r0 = 1 + rg * RROWS
                    bps = ps_b.tile([128, NT], F32)
                    for d in range(3):
                        dy = d - 1
                        nc.tensor.matmul(
                            bps[:],
                            ones3[:],
                            a3[:, r0 + dy : r0 + dy + RROWS, 1 : 1 + W],
                            start=(d == 0),
                            stop=(d == 2),
                        )
                    for k in range(KCH):
                        nc.vector.tensor_mul(
                            ot[:, k, rg * NT : (rg + 1) * NT],
                            xo[:, k, rg * NT : (rg + 1) * NT],
                            bps[:],
                        )
                        if rg % 2 == 1 or rg == RG - 1:
                            st0 = (rg - 1 if rg % 2 == 1 else rg) * NT
                            nc.gpsimd.dma_start(
                                out_d[img, k, :, st0 : (rg + 1) * NT],
                                ot[:, k, st0 : (rg + 1) * NT],
                            )

            # 1-image skew: conv2/bcast of img-1 interleaves with conv1 of img
            stage_front(0)
            for img in range(1, BL):
                stage_front(img)
                stage_back(img - 1)
            stage_back(BL - 1)

    nc.compile()
    return nc


def _prep_shards(x_in, x_out, w1, w2):
    bf16 = ml_dtypes.bfloat16
    fp8 = ml_dtypes.float8_e4m3
    # w1t[t, c, k, m] = w1[m, 128k + c, dy, dx],  t = dy*3 + dx
    w1t = np.ascontiguousarray(
        w1.reshape(16, KCH, 128, 9).transpose(3, 2, 1, 0)
    ).astype(fp8)
    w2t = np.zeros((96, 3), dtype=bf16)
    # w2t[32*j + c, d] = w2[0, c, d, j]   (j = dx index, d = dy index)
    for j in range(3):
        w2t[32 * j : 32 * j + 16, :] = w2[0, :, :, j].astype(bf16)
    xi = np.zeros((NCORES, BL, KCH, 128, HP, HP), dtype=fp8)
    xi[..., 1 : 1 + H, 1 : 1 + W] = (
        x_in.reshape(NCORES, BL, KCH, 128, H, W).astype(fp8)
    )
    xi = xi.reshape(NCORES, BL, KCH, 128, HP * HP)
    xo = x_out.reshape(NCORES, BL, KCH, 128, HW).astype(bf16)
    return [
        {
            "xin": np.ascontiguousarray(xi[i]),
            "xout": np.ascontiguousarray(xo[i]),
            "w1t": w1t,
            "w2t": w2t,
        }
        for i in range(NCORES)
    ]


def _run(in_maps, trace=False):
    if "nc" not in _cache:
        _cache["nc"] = _build()
    return run_bass_kernel_spmd(
        _cache["nc"], in_maps, core_ids=list(range(NCORES)), trace=trace
    )


def kernel(x_in, x_out, w1, w2, _trace=False):
    in_maps = _prep_shards(
        np.asarray(x_in, dtype=np.float32),
        np.asarray(x_out, dtype=np.float32),
        np.asarray(w1, dtype=np.float32),
        np.asarray(w2, dtype=np.float32),
    )
    res = _run(in_maps, trace=_trace)
    out = np.stack([res.results[i]["out"] for i in range(NCORES)])
    kernel.last_exec_time_ns = res.exec_time_ns
    return out.reshape(B, C, H, W).astype(np.float32)



# revision 5
# speedup vs baseline: 1.0606x; 1.0606x over previous
"""ALayer kernel for 8 TRN2 NeuronCores — pure data parallel over batch.

Per-core shard: 4 images of [256, 56, 56].
  h  = relu(conv3x3(x_in, w1))      # 256 -> 16 ch
  A  = sigmoid(conv3x3(h, w2))      # 16 -> 1 ch
  out = x_out * box3x3(A)           # broadcast over 256 ch

v2 design — column-tiled TensorEngine (4 concurrent 32-col subarray strips)
with an interleaved row-phase layout:
  Data row y (padded coords 1..56) maps to phase j=(y-1)%4, supergroup
  s=(y-1)//28, slot r=((y-1)%28)//4, i.e. y = 1 + 28s + 4r + j.
  conv1: per supergroup s, 18 accumulation rounds (9 taps x 2 K-chunks of
         128 in-ch); each round issues 4 concurrent col-tiled matmuls
         (tile_position=(0,32j)), M=16 out-ch, N=7x56 pixels of phase j.
  relu:  one [128,7,56] PSUM->SBUF activation per supergroup into h_tmp
         [128, 16, 58] (partition 32j+m = (phase j, ch m); slot/col guards
         zero) — conv2 reads h_tmp directly, no shuffle needed.
  conv2: per supergroup, 9 rounds (dy,dx) x 4 col-tiled K=16 matmuls from
         h_tmp group g=(j+dy-1)%4 with slot carry; M=1 at psum[32j].
  sigmoid: one [128,7,56] activation per supergroup -> a_tmp.
  a9:    9 pre-shifted copies of the A plane: 3 scatter-DMAs (dx copies
         into a9[3:6]) + 2 row-shift DMAs (a9[0:3], a9[6:9]).
  box:   ONE K=9 matmul per 7-row group (lhsT = ones[9,128]) produces
         box3x3(A) broadcast to 128 partitions in PSUM.
  mul:   DVE multiply with x_out (bf16 out, cast to fp32 on host).
"""

import numpy as np
import ml_dtypes

import concourse.bass as bass
import concourse.tile as tile
import concourse.mybir as mybir
from concourse import bacc
from concourse.bass_utils import run_bass_kernel_spmd

BF16 = mybir.dt.bfloat16
FP8 = mybir.dt.float8e4
F32 = mybir.dt.float32

B, C, H, W = 32, 256, 56, 56
NCORES = 8
BL = B // NCORES          # images per core
KCH = 2                   # 256 = 2 chunks of 128
HP = H + 2                # padded plane side (58)
HW = H * W                # 3136
PL = HP * HP              # 3364

_cache = {}


def _build():
    nc = bacc.Bacc("TRN2", target_bir_lowering=False, debug=False)

    xin_d = nc.dram_tensor("xin", [BL, KCH, 128, PL], FP8, kind="ExternalInput").ap()
    xout_d = nc.dram_tensor("xout", [BL, 128, KCH, HW], BF16, kind="ExternalInput").ap()
    w1_d = nc.dram_tensor("w1t", [128, 9, KCH, 16], FP8, kind="ExternalInput").ap()
    w2_d = nc.dram_tensor("w2t", [128, 9], BF16, kind="ExternalInput").ap()
    out_d = nc.dram_tensor("out", [BL, 128, KCH, HW], BF16, kind="ExternalOutput").ap()

    with tile.TileContext(nc) as tc:
        with (
            tc.tile_pool(name="const", bufs=1) as constp,
            tc.tile_pool(name="xpad", bufs=2) as xpadp,
            tc.tile_pool(name="ht", bufs=2) as htp,
            tc.tile_pool(name="h1", bufs=3) as h1p,
            tc.tile_pool(name="at", bufs=2) as atp,
            tc.tile_pool(name="a9", bufs=3) as a9p,
            tc.tile_pool(name="xo", bufs=3) as xop,
            tc.tile_pool(name="ot", bufs=2) as otp,
            tc.tile_pool(name="ps_h", bufs=2, space="PSUM") as ps_h,
            tc.tile_pool(name="ps_a", bufs=2, space="PSUM") as ps_a,
            tc.tile_pool(name="ps_b", bufs=3, space="PSUM") as ps_b,
        ):
            w1sb = constp.tile([128, 9, KCH, 16], FP8)
            w2sb = constp.tile([128, 9], BF16)
            nc.scalar.dma_start(w1sb[:], w1_d[:])
            nc.scalar.dma_start(w2sb[:], w2_d[:])
            ones9 = constp.tile([9, 128], BF16)
            nc.vector.memset(ones9[:], 1.0)

            h1s, a9s, xos = {}, {}, {}

            def stage_front(img):
                # ---- load x_in (pre-padded fp8), split rows 0-29 / 30-57 ----
                xpad = xpadp.tile([128, KCH, HP, HP], FP8)
                xpf = xpad.rearrange("p k r w -> p k (r w)")
                MID = 30 * HP
                for k in range(KCH):
                    nc.sync.dma_start(xpf[:, k, 0:MID], xin_d[img, k, :, 0:MID])
                for k in range(KCH):
                    nc.sync.dma_start(xpf[:, k, MID:PL], xin_d[img, k, :, MID:PL])

                # ---- conv1 (+relu) -> h_tmp, col-tiled over 4 row phases ----
                ht = htp.tile([128, 14, HP], BF16)
                if img < 2:
                    # zero col guards 0, 57
                    nc.gpsimd.memset(ht[:, :, 0], 0.0)
                    nc.gpsimd.memset(ht[:, :, 57], 0.0)
                for s in range(2):
                    ps = ps_h.tile([128, 7, 56], F32)
                    rnd = 0
                    for t in range(9):
                        dy, dx = t // 3, t % 3
                        for k in range(KCH):
                            for j in range(4):
                                rs = 28 * s + j + dy
                                nc.tensor.matmul(
                                    ps[32 * j : 32 * j + 16],
                                    w1sb[:, t, k, :],
                                    xpad[:, k, rs : rs + 25 : 4, dx : dx + 56],
                                    start=(rnd == 0),
                                    stop=(rnd == 17),
                                    tile_position=(0, 32 * j),
                                    skip_group_check=True,
                                )
                            rnd += 1
                    nc.scalar.activation(
                        ht[:, 7 * s : 7 * s + 7, 1:57],
                        ps[:],
                        mybir.ActivationFunctionType.Relu,
                    )

                # ---- h-scatter: h_tmp -> h1 plane (4 DMAs, one per phase) ----
                h1 = h1p.tile([16, HP, HP], BF16)
                h1s[img] = h1
                if img < 3:
                    nc.vector.memset(h1[:, 0, :], 0.0)
                    nc.vector.memset(h1[:, 57, :], 0.0)
                for j in range(4):
                    # dst rows y = 1 + 28s + 4r + j, full 58-wide rows
                    dst = h1[:, 1 + j : 1 + j + 53 : 4, :]
                    nc.gpsimd.dma_start(dst, ht[32 * j : 32 * j + 16])

                # prefetch x_out
                xo = xop.tile([128, KCH, HW], BF16)
                xos[img] = xo
                nc.gpsimd.dma_start(xo[:], xout_d[img])

            def stage_mid(img):
                h1 = h1s[img]
                at = atp.tile([128, 2, 7, HP], BF16)
                if img < 2:
                    nc.vector.memset(at[:, :, :, 0], 0.0)
                    nc.vector.memset(at[:, :, :, 57], 0.0)
                for s in range(2):
                    ps = ps_a.tile([128, 7, 56], F32)
                    rnd = 0
                    for dy in range(3):
                        for dx in range(3):
                            for j in range(4):
                                b = 4 * s + j
                                nc.tensor.matmul(
                                    ps[32 * j : 32 * j + 1],
                                    w2sb[0:16, rnd : rnd + 1],
                                    h1[:, 7 * b + dy : 7 * b + dy + 7, dx : dx + 56],
                                    start=(rnd == 0),
                                    stop=(rnd == 8),
                                    tile_position=(0, 32 * j),
                                    skip_group_check=True,
                                )
                            rnd += 1
                    nc.scalar.activation(
                        at[:, s, :, 1:57],
                        ps[:],
                        mybir.ActivationFunctionType.Sigmoid,
                    )

                # ---- build a9: 6 run-scatters into a9[3:6], 2 row-shift fills ----
                a9 = a9p.tile([9, HP, HP], BF16)
                a9s[img] = a9
                if img < 3:
                    nc.vector.memset(a9[:, 0, :], 0.0)
                    nc.vector.memset(a9[:, 57, :], 0.0)
                    nc.vector.memset(a9[:, :, 0:2], 0.0)
                    nc.vector.memset(a9[:, :, 56:58], 0.0)
                a9f = a9.rearrange("p r w -> p (r w)")
                for c in range(3):
                    for s in range(2):
                        # contiguous 1624-run; 1-elem overflows land on guards
                        st = (1 + 28 * s) * HP + (1 - c)
                        nc.gpsimd.dma_start(
                            a9f[3 + c : 4 + c, st : st + 1624],
                            at[0:128:32, s],
                        )
                nc.gpsimd.dma_start(a9f[0:3, HP : 57 * HP], a9f[3:6, 0 : 56 * HP])
                nc.gpsimd.dma_start(a9f[6:9, HP : 57 * HP], a9f[3:6, 2 * HP : PL])

            def stage_back(img):
                a9 = a9s[img]
                xo = xos[img]
                ot = otp.tile([128, KCH, HW], BF16)
                for R in range(8):
                    ps = ps_b.tile([128, 7, 56], F32)
                    nc.tensor.matmul(
                        ps[:],
                        ones9[:],
                        a9[:, 1 + 7 * R : 8 + 7 * R, 1:57],
                        start=True,
                        stop=True,
                    )
                    psf = ps.rearrange("p r w -> p (r w)")
                    for k in range(KCH):
                        nc.vector.tensor_mul(
                            ot[:, k, 392 * R : 392 * (R + 1)],
                            xo[:, k, 392 * R : 392 * (R + 1)],
                            psf[:],
                        )
                for k in range(KCH):
                    nc.scalar.dma_start(out_d[img, :, k, :], ot[:, k, :])

            # software pipeline: F(i) || M(i-1) || B(i-2)
            stage_front(0)
            stage_front(1)
            stage_mid(0)
            stage_front(2)
            stage_mid(1)
            stage_back(0)
            stage_front(3)
            stage_mid(2)
            stage_back(1)
            stage_mid(3)
            stage_back(2)
            stage_back(3)

    nc.compile()
    return nc


def _prep_shards(x_in, x_out, w1, w2):
    bf16 = ml_dtypes.bfloat16
    fp8 = ml_dtypes.float8_e4m3
    # w1t[c, t, k, m] = w1[m, 128k + c, dy, dx],  t = 3*dy + dx
    w1t = np.ascontiguousarray(
        w1.reshape(16, KCH, 128, 9).transpose(2, 3, 1, 0)
    ).astype(fp8)
    # w2t[32g + c, t] = w2[0, c, dy, dx] replicated at 4 partition bases
    w2t = np.zeros((128, 9), dtype=bf16)
    for g in range(4):
        w2t[32 * g : 32 * g + 16, :] = w2[0].reshape(16, 9).astype(bf16)
    xi = np.zeros((NCORES, BL, KCH, 128, HP, HP), dtype=fp8)
    xi[..., 1 : 1 + H, 1 : 1 + W] = (
        x_in.reshape(NCORES, BL, KCH, 128, H, W).astype(fp8)
    )
    xi = xi.reshape(NCORES, BL, KCH, 128, PL)
    # xout[img, c_partition, k, hw]
    xo = np.ascontiguousarray(
        x_out.reshape(NCORES, BL, KCH, 128, HW).transpose(0, 1, 3, 2, 4)
    ).astype(bf16)
    return [
        {
            "xin": np.ascontiguousarray(xi[i]),
            "xout": xo[i],
            "w1t": w1t,
            "w2t": w2t,
        }
        for i in range(NCORES)
    ]


def _run(in_maps, trace=False):
    if "nc" not in _cache:
        _cache["nc"] = _build()
    return run_bass_kernel_spmd(
        _cache["nc"], in_maps, core_ids=list(range(NCORES)), trace=trace
    )


def kernel(x_in, x_out, w1, w2, _trace=False):
    in_maps = _prep_shards(
        np.asarray(x_in, dtype=np.float32),
        np.asarray(x_out, dtype=np.float32),
        np.asarray(w1, dtype=np.float32),
        np.asarray(w2, dtype=np.float32),
    )
    res = _run(in_maps, trace=_trace)
    # out[img, c_partition, k, hw] bf16 -> [B, C, H, W] fp32
    out = np.stack([res.results[i]["out"] for i in range(NCORES)])
    kernel.last_exec_time_ns = res.exec_time_ns
    out = out.astype(np.float32).transpose(0, 1, 3, 2, 4)
    return out.reshape(B, C, H, W)
